# revision 1
# baseline (speedup 1.0000x reference)
"""Trainium2 Bass kernel for DFBNet SSP (sparse_attention).

Data-parallel over batch: 8 samples -> 8 NeuronCores, one sample per core.

Per-sample device computation (all heavy tensor work):
  - FP (masked avg-pool of support feat) and fg/bg prototypes of feature_q
  - column norms of feature_q, normalized cn
  - sim = 2 * cn.T @ cn                               [N,N] gram matmul
  - T[k,n] = wb[k] * exp(sim[k,n])  (additive -BIG mask fused into Exp bias)
  - colsum[n] = sum_k T[k,n] (== softmax row-sums by symmetry of sim)
  - bg_local[c,n] = sum_k fq[c,k] T[k,n] / colsum[n]  (== (bg_attn @ cur.T).T)
  - BP1 ~ bg_proto*(3/7) + bg_local, FP1 ~ FP + fg_proto (cosine is
    scale-invariant so the reference's 0.3/0.7 and 0.5/0.5 blends are applied
    up to a positive scale that cancels)
  - out = 10 * cosine(feature_q, {BP1, FP1}) along C

Host side computes only the {0,1} threshold-selection vectors wf/wb (float64
replica of the reference pred chain incl. the top-k fallback).  These are
discrete bits whose exact values a device fp32 pipeline could flip at
~1e-7-margin pixels, with O(1) output impact; everything continuous stays on
device.
"""

import numpy as np

B, C, H, W = 8, 512, 32, 32
N = H * W
FG_THRES, BG_THRES, TOPK = 0.7, 0.6, 12
BIG = 60000.0
LN10 = 2.302585092994046  # additive pre-exp mask; exp(x - BIG) == 0.0 in fp32

CC = C // 128  # 4 channel chunks
KC = N // 128  # 8 pixel chunks
NB = N // 512  # 2 psum-bank column groups

_cache = {}
_EYE = np.eye(128, dtype=np.float32)


# --------------------------------------------------------------------------
# host: selection weights (exact reference semantics, float64)
# --------------------------------------------------------------------------
def _host_select_weights(feature_q, support_feat, support_mask):
    fq = feature_q.astype(np.float64).reshape(B, C, N)
    sf = support_feat.astype(np.float64).reshape(B, C, N)
    mf = (support_mask.reshape(B, N) == 1).astype(np.float64)
    mb = 1.0 - mf
    FP = (sf * mf[:, None]).sum(-1) / (mf.sum(-1)[:, None] + 1e-5)
    BP = (sf * mb[:, None]).sum(-1) / (mb.sum(-1)[:, None] + 1e-5)

    def cos(a, b):  # a [B,C,N], b [B,C]
        dot = (a * b[:, :, None]).sum(1)
        na = np.sqrt((a * a).sum(1))
        nb = np.sqrt((b * b).sum(1))[:, None]
        return dot / np.maximum(na * nb, 1e-8)

    sfg = cos(fq, FP) * 10.0
    sbg = cos(fq, BP) * 10.0
    m = np.maximum(sfg, sbg)
    efg = np.exp(sfg - m)
    ebg = np.exp(sbg - m)
    pfg = efg / (efg + ebg)
    pbg = ebg / (efg + ebg)

    def select(pred, thres):
        w = np.zeros((B, N), np.float32)
        for b in range(B):
            row = pred[b] > thres
            if row.sum() > 0:
                w[b] = row
            else:
                # jax.lax.top_k tie-break: lower index wins -> stable argsort
                idx = np.argsort(-pred[b], kind="stable")[:TOPK]
                w[b, idx] = 1.0
        return w

    return select(pfg, FG_THRES), select(pbg, BG_THRES)


# --------------------------------------------------------------------------
# device program
# --------------------------------------------------------------------------
def _make_tile_context_cls():
    import concourse.tile as tile
    from concourse.vector_clock import ScopedClock, VectorClock

    class PatchedTileContext(tile.TileContext):
        """This walrus build rejects CTRL/Drain instructions carrying more
        than one sem wait.  Put the tail-drain's global-clock waits on
        single-wait NOPs (same engine, program order) instead."""

        def _drain_and_barrier(self, tick_clock, wait_clock):
            gc = tick_clock.global_clock
            n = len(gc)
            for proc in range(n):
                t = gc[proc]
                if t > 0:
                    vec = [0] * n
                    vec[proc] = t
                    nop = self.nc.sync.nop(nofuse=True)
                    wait_clock.add_sem_waits(
                        nop.ins, ScopedClock({None: VectorClock(vec)})
                    )
            self.nc.sync.drain()
            self.nc.all_engine_barrier()
            assert self.sems is not None
            popped = self.nc._tile_sem_poison_stack.pop()
            assert popped is self._sem_poison
            self.nc.clear_and_free_semaphores(list(self.sems.allocated().values()))
            self.nc.all_engine_barrier()

    return PatchedTileContext


def _split_multi_waits(nc):
    """This walrus build allows at most one sync-wait command per
    instruction.  Move extra waits onto same-engine NOPs inserted just
    before the instruction (waits are AND conditions; order-safe)."""
    import concourse.mybir as mybir

    n_split = 0
    for f in nc.m.functions:
        for bb in f.blocks:
            il = bb.instructions
            i = 0
            while i < len(il):
                inst = il[i]
                si = inst.sync_info
                if si is not None and si.on_wait and len(si.on_wait) > 1:
                    waits = list(si.on_wait)
                    for j, w in enumerate(waits[:-1]):
                        nop = mybir.InstNoOp(
                            name=f"{inst.name}-wsplit{j}",
                            ins=[],
                            outs=[],
                            engine=inst.engine,
                            sync_info=mybir.SyncInfo(on_wait=[w], on_update=[]),
                        )
                        il.insert(i, nop)
                        i += 1
                        n_split += 1
                    inst.sync_info = mybir.SyncInfo(
                        on_wait=[waits[-1]], on_update=si.on_update
                    )
                i += 1
    return n_split


def _build_nc(split_waits=True):
    import concourse.bass as bass
    import concourse.mybir as mybir

    fp32 = mybir.dt.float32
    f32r = mybir.dt.float32r
    AF = mybir.ActivationFunctionType
    ALU = mybir.AluOpType
    AX = mybir.AxisListType

    PatchedTileContext = _make_tile_context_cls()

    nc = bass.Bass("TRN2", target_bir_lowering=False)
    fq_d = nc.declare_dram_parameter("fq", [C, N], fp32, isOutput=False)
    id_d = nc.declare_dram_parameter("ident", [128, 128], fp32, isOutput=False)
    sf_d = nc.declare_dram_parameter("sf", [C, N], fp32, isOutput=False)
    mf_d = nc.declare_dram_parameter("mf", [1, N], fp32, isOutput=False)
    wf_d = nc.declare_dram_parameter("wf", [1, N], fp32, isOutput=False)
    wb_d = nc.declare_dram_parameter("wb", [1, N], fp32, isOutput=False)
    out_d = nc.declare_dram_parameter("out", [2, N], fp32, isOutput=True)

    def nbs(nb):
        return slice(nb * 512, (nb + 1) * 512)

    with PatchedTileContext(nc) as tc:
        with (
            tc.tile_pool(name="consts", bufs=1) as consts,
            tc.tile_pool(name="big", bufs=1) as big,
            tc.tile_pool(name="scr", bufs=2) as scr,
            tc.tile_pool(name="small", bufs=1) as small,
        ):
            # ---- constants / small inputs
            ident = consts.tile([128, 128], fp32, tag="ident")
            nc.sync.dma_start(ident, id_d[:, :])
            ones_f = consts.tile([128, 128], fp32, tag="ones_f")
            nc.vector.memset(ones_f, 1.0)
            ones = consts.tile([128, 128], f32r, tag="ones")
            nc.vector.tensor_copy(ones, ones_f)

            ln10c = consts.tile([1, 1], fp32, tag="ln10c")
            nc.vector.memset(ln10c, LN10)
            mfrow = consts.tile([1, N], fp32, tag="mfrow")
            nc.sync.dma_start(mfrow, mf_d[:, :])
            wfrow = consts.tile([1, N], fp32, tag="wfrow")
            nc.sync.dma_start(wfrow, wf_d[:, :])
            wbrow = consts.tile([1, N], fp32, tag="wbrow")
            nc.sync.dma_start(wbrow, wb_d[:, :])
            wbcol = consts.tile([128, KC], fp32, tag="wbcol")
            nc.sync.dma_start(wbcol, wb_d[0, :].rearrange("(a b) -> b a", b=128))
            # bias = (wb - 1) * BIG  ->  {0 -> -BIG, 1 -> 0}
            biascol = consts.tile([128, KC], fp32, tag="biascol")
            nc.vector.tensor_scalar(
                biascol, wbcol, BIG, BIG, op0=ALU.mult, op1=ALU.subtract
            )

            # ---- main inputs
            fq = []
            sfc = []
            for cc in range(CC):
                t = big.tile([128, N], fp32, tag=f"fq{cc}", name=f"fqs{cc}")
                nc.sync.dma_start(t, fq_d[cc * 128 : (cc + 1) * 128, :])
                fq.append(t)
            for cc in range(CC):
                t = big.tile([128, N], fp32, tag=f"sf{cc}", name=f"sfs{cc}")
                nc.sync.dma_start(t, sf_d[cc * 128 : (cc + 1) * 128, :])
                sfc.append(t)

            # f32r row copies (broadcast matmul operands; 0/1 exact in f32r)
            mfrow_r = consts.tile([1, N], f32r, tag="mfrow_r")
            nc.vector.tensor_copy(mfrow_r, mfrow)
            wfrow_r = consts.tile([1, N], f32r, tag="wfrow_r")
            nc.vector.tensor_copy(wfrow_r, wfrow)
            wbrow_r = consts.tile([1, N], f32r, tag="wbrow_r")
            nc.vector.tensor_copy(wbrow_r, wbrow)
            # ---- mask broadcasts [128, N] via K=1 ones-matmul (PSUM) + copy
            mfB = consts.tile([128, N], fp32, tag="mfB")
            wfB = consts.tile([128, N], fp32, tag="wfB")
            wbB = consts.tile([128, N], fp32, tag="wbB")

            # ---- transposes (PE) + column norms
            fqT = [big.tile([128, C], f32r, tag=f"fqT{kc}", name=f"fqT{kc}") for kc in range(KC)]
            na2row = consts.tile([1, N], fp32, tag="na2row")
            rnormB = big.tile([128, N], fp32, tag="rnormB")
            with tc.tile_pool(name="ps_pre", bufs=2, space="PSUM") as ps_pre:
                for row, dst in ((mfrow_r, mfB), (wfrow_r, wfB), (wbrow_r, wbB)):
                    for nb in range(NB):
                        bc = ps_pre.tile([128, 512], fp32, tag="bc", name="bc")
                        nc.tensor.matmul(
                            bc, ones[0:1, :], row[:, nbs(nb)], start=True, stop=True
                        )
                        nc.scalar.copy(dst[:, nbs(nb)], bc)
                for kc in range(KC):
                    trp = ps_pre.tile([128, 512], fp32, tag="tr", name=f"trp{kc}")
                    for cc in range(CC):
                        nc.tensor.transpose(
                            trp[:, cc * 128 : (cc + 1) * 128],
                            fq[cc][:, kc * 128 : (kc + 1) * 128],
                            ident,
                        )
                    nc.scalar.copy(fqT[kc], trp)

                n2ps = [ps_pre.tile([128, 512], fp32, tag="n2", name=f"n2ps{nb}") for nb in range(NB)]
                for cc in range(CC):
                    sq = scr.tile([128, N], f32r, tag="sqr", bufs=2, name="sq")
                    nc.vector.tensor_mul(sq, fq[cc], fq[cc])
                    for nb in range(NB):
                        nc.tensor.matmul(
                            n2ps[nb],
                            ones,
                            sq[:, nbs(nb)],
                            start=(cc == 0),
                            stop=(cc == CC - 1),
                        )
                tmp = scr.tile([128, N], fp32, tag="scr")
                for nb in range(NB):
                    nc.vector.tensor_copy(na2row[:, nbs(nb)], n2ps[nb][0:1, :])
                    nc.scalar.activation(tmp[:, nbs(nb)], n2ps[nb], AF.Ln)
                nc.scalar.activation(rnormB, tmp, AF.Exp, scale=-0.5)

            # ---- cn = fq * rnormB
            cn = []
            for cc in range(CC):
                t = big.tile([128, N], f32r, tag=f"cn{cc}", name=f"cns{cc}")
                nc.vector.tensor_mul(t, fq[cc], rnormB)
                cn.append(t)

            # ---- prototypes (free-dim masked reductions on DVE)
            FPr = small.tile([128, CC], fp32, tag="FPr")
            FGr = small.tile([128, CC], fp32, tag="FGr")
            BGr = small.tile([128, CC], fp32, tag="BGr")
            # gpsimd is otherwise idle and these are off the critical path
            for cc in range(CC):
                for acc, a, b in (
                    (FPr, sfc[cc], mfB),
                    (FGr, fq[cc], wfB),
                    (BGr, fq[cc], wbB),
                ):
                    o = scr.tile([128, N], fp32, tag="gscr", bufs=2, name="ttro")
                    nc.gpsimd.tensor_mul(o, a, b)
                    snk = scr.tile([128, N], fp32, tag="scr", name="snk")
                    nc.scalar.activation(
                        snk, o, AF.Copy, accum_out=acc[:, cc : cc + 1]
                    )
            cntm = small.tile([128, 1], fp32, tag="cntm")
            nc.vector.reduce_sum(cntm, mfB, axis=AX.X)
            cntf = small.tile([128, 1], fp32, tag="cntf")
            nc.vector.reduce_sum(cntf, wfB, axis=AX.X)
            cntb = small.tile([128, 1], fp32, tag="cntb")
            nc.vector.reduce_sum(cntb, wbB, axis=AX.X)

            rcntm = small.tile([128, 1], fp32, tag="rcntm")
            nc.vector.tensor_scalar_add(rcntm, cntm, 1e-5)
            nc.vector.reciprocal(rcntm, rcntm)
            rcntf = small.tile([128, 1], fp32, tag="rcntf")
            nc.vector.reciprocal(rcntf, cntf)
            rcntb = small.tile([128, 1], fp32, tag="rcntb")
            nc.vector.reciprocal(rcntb, cntb)
            nc.vector.tensor_scalar_mul(rcntb, rcntb, 3.0 / 7.0)

            # FP1 ~ FP + fg_proto  (2*FP_1 of the reference; scale cancels)
            FP1 = small.tile([128, CC], fp32, tag="FP1")
            nc.vector.tensor_scalar_mul(FP1, FPr, rcntm)
            tmp4 = small.tile([128, CC], fp32, tag="tmp4")
            nc.vector.tensor_scalar_mul(tmp4, FGr, rcntf)
            nc.vector.tensor_add(FP1, FP1, tmp4)
            # bgp_s = (3/7) * bg_proto
            bgp_s = small.tile([128, CC], fp32, tag="bgp_s")
            nc.vector.tensor_scalar_mul(bgp_s, BGr, rcntb)

            # ---- gram + exp + colsum + bg reconstruction
            T = [big.tile([128, N], f32r, tag=f"T{kc}", name=f"T{kc}") for kc in range(KC)]
            rcolB = big.tile([128, N], fp32, tag="rcolB")
            BP1 = [big.tile([128, N], fp32, tag=f"BP1{cc}", name=f"BP1_{cc}") for cc in range(CC)]
            with (
                tc.tile_pool(name="ps_sim", bufs=4, space="PSUM") as ps_sim,
                tc.tile_pool(name="ps_cs", bufs=2, space="PSUM") as ps_cs,
                tc.tile_pool(name="ps_bg", bufs=2, space="PSUM") as ps_bg,
            ):
                csps = [ps_cs.tile([128, 512], fp32, tag="cs", name=f"csps{nb}") for nb in range(NB)]
                for mi in range(KC):
                    for nb in range(NB):
                        simp = ps_sim.tile([128, 512], fp32, tag="sim", name=f"simp{mi}_{nb}")
                        for cc in range(CC):
                            nc.tensor.matmul(
                                simp,
                                cn[cc][:, mi * 128 : (mi + 1) * 128],
                                cn[cc][:, nbs(nb)],
                                start=(cc == 0),
                                stop=(cc == CC - 1),
                            )
                        nc.scalar.activation(
                            T[mi][:, nbs(nb)],
                            simp,
                            AF.Exp,
                            bias=biascol[:, mi : mi + 1],
                            scale=2.0,
                        )
                        nc.tensor.matmul(
                            csps[nb],
                            ones,
                            T[mi][:, nbs(nb)],
                            start=(mi == 0),
                            stop=(mi == KC - 1),
                        )
                tmpc = scr.tile([128, N], fp32, tag="scr")
                for nb in range(NB):
                    nc.scalar.activation(tmpc[:, nbs(nb)], csps[nb], AF.Ln)
                nc.scalar.activation(rcolB, tmpc, AF.Exp, scale=-1.0)

                for mi2 in range(CC):
                    bgp = [ps_bg.tile([128, 512], fp32, tag="bg", name=f"bgp{mi2}_{nb}") for nb in range(NB)]
                    for kc in range(KC):
                        for nb in range(NB):
                            nc.tensor.matmul(
                                bgp[nb],
                                fqT[kc][:, mi2 * 128 : (mi2 + 1) * 128],
                                T[kc][:, nbs(nb)],
                                start=(kc == 0),
                                stop=(kc == KC - 1),
                            )
                    for nb in range(NB):
                        nc.vector.tensor_mul(
                            BP1[mi2][:, nbs(nb)], bgp[nb], rcolB[:, nbs(nb)]
                        )
                    nc.vector.tensor_scalar_add(
                        BP1[mi2], BP1[mi2], bgp_s[:, mi2 : mi2 + 1]
                    )

            # ---- final similarities
            with tc.tile_pool(name="ps_fin", bufs=1, space="PSUM") as ps_fin:
                dfg = [ps_fin.tile([1, 512], fp32, tag=f"dfg{nb}", name=f"dfg{nb}") for nb in range(NB)]
                for cc in range(CC):
                    for nb in range(NB):
                        nc.tensor.matmul(
                            dfg[nb],
                            FP1[:, cc : cc + 1],
                            fq[cc][:, nbs(nb)],
                            start=(cc == 0),
                            stop=(cc == CC - 1),
                        )
                sqf = small.tile([128, CC], fp32, tag="sqf")
                nc.vector.tensor_mul(sqf, FP1, FP1)
                rsum = small.tile([128, 1], fp32, tag="rsum")
                nc.vector.reduce_sum(rsum, sqf, axis=AX.X)
                nfps = ps_fin.tile([1, 1], fp32, tag="nfp2")
                nc.tensor.matmul(nfps, ones_f[:, 0:1], rsum, start=True, stop=True)
                nfp2s = small.tile([1, 1], fp32, tag="nfp2s")
                nc.vector.tensor_copy(nfp2s, nfps)

                dbg = [ps_fin.tile([1, 512], fp32, tag=f"dbg{nb}", name=f"dbg{nb}") for nb in range(NB)]
                qps = [ps_fin.tile([1, 512], fp32, tag=f"q{nb}", name=f"qps{nb}") for nb in range(NB)]
                for cc in range(CC):
                    p_t = scr.tile([128, N], f32r, tag="sqr", bufs=2, name="p_t")
                    nc.vector.tensor_mul(p_t, fq[cc], BP1[cc])
                    q_t = scr.tile([128, N], f32r, tag="sqr", bufs=2, name="q_t")
                    nc.vector.tensor_mul(q_t, BP1[cc], BP1[cc])
                    for nb in range(NB):
                        nc.tensor.matmul(
                            dbg[nb],
                            ones[:, 0:1],
                            p_t[:, nbs(nb)],
                            start=(cc == 0),
                            stop=(cc == CC - 1),
                        )
                        nc.tensor.matmul(
                            qps[nb],
                            ones[:, 0:1],
                            q_t[:, nbs(nb)],
                            start=(cc == 0),
                            stop=(cc == CC - 1),
                        )

                # final rows: two separate [1,N] chains (partition 0 only)
                dotfg_s = small.tile([1, N], fp32, tag="rowtmp", bufs=5, name="dotfg_s")
                for nb in range(NB):
                    nc.vector.tensor_copy(dotfg_s[:, nbs(nb)], dfg[nb])
                prodfg = small.tile([1, N], fp32, tag="rowtmp", bufs=5, name="prodfg")
                nc.scalar.mul(prodfg, na2row, nfp2s)
                nc.vector.tensor_scalar(prodfg, prodfg, 1e-16, None, op0=ALU.max)
                nc.scalar.activation(prodfg, prodfg, AF.Ln)
                nc.scalar.activation(prodfg, prodfg, AF.Exp, scale=-0.5, bias=ln10c)
                outfg = small.tile([1, N], fp32, tag="rowtmp", bufs=5, name="outfg")
                nc.vector.tensor_mul(outfg, dotfg_s, prodfg)
                nc.sync.dma_start(out_d[1:2, :], outfg)

                dotbg_s = small.tile([1, N], fp32, tag="rowtmp", bufs=5, name="dotbg_s")
                nb2bg = small.tile([1, N], fp32, tag="rowtmp", bufs=5, name="nb2bg")
                for nb in range(NB):
                    nc.vector.tensor_copy(dotbg_s[:, nbs(nb)], dbg[nb])
                    nc.vector.tensor_copy(nb2bg[:, nbs(nb)], qps[nb])
                prodbg = small.tile([1, N], fp32, tag="rowtmp", bufs=5, name="prodbg")
                nc.vector.tensor_mul(prodbg, na2row, nb2bg)
                nc.vector.tensor_scalar(prodbg, prodbg, 1e-16, None, op0=ALU.max)
                nc.scalar.activation(prodbg, prodbg, AF.Ln)
                nc.scalar.activation(prodbg, prodbg, AF.Exp, scale=-0.5, bias=ln10c)
                outbg = small.tile([1, N], fp32, tag="rowtmp", bufs=5, name="outbg")
                nc.vector.tensor_mul(outbg, dotbg_s, prodbg)
                nc.sync.dma_start(out_d[0:1, :], outbg)

    if split_waits:
        _split_multi_waits(nc)
    return nc


def _get_nc():
    if "nc" not in _cache:
        _cache["nc"] = _build_nc()
    return _cache["nc"]


def _make_in_maps(feature_q, support_feat, support_mask):
    wf, wb = _host_select_weights(feature_q, support_feat, support_mask)
    fqr = np.ascontiguousarray(feature_q.reshape(B, C, N), dtype=np.float32)
    sfr = np.ascontiguousarray(support_feat.reshape(B, C, N), dtype=np.float32)
    mfr = (support_mask.reshape(B, N) == 1).astype(np.float32)
    return [
        {
            "fq": fqr[b],
            "ident": _EYE,
            "sf": sfr[b],
            "mf": mfr[b : b + 1],
            "wf": wf[b : b + 1],
            "wb": wb[b : b + 1],
        }
        for b in range(B)
    ]


def run_sharded(feature_q, support_feat, support_mask, **kwargs):
    """Run on all 8 cores; returns (output [B,2,H,W], BassKernelResults)."""
    from concourse.bass_utils import run_bass_kernel_spmd

    nc = _get_nc()
    in_maps = _make_in_maps(feature_q, support_feat, support_mask)
    res = run_bass_kernel_spmd(nc, in_maps, core_ids=list(range(B)), **kwargs)
    out = np.stack([res.results[b]["out"] for b in range(B)])
    return out.reshape(B, 2, H, W).astype(np.float32), res


def kernel(feature_q, support_feat, support_mask):
    out, _ = run_sharded(
        np.asarray(feature_q), np.asarray(support_feat), np.asarray(support_mask)
    )
    return out



# revision 7
# speedup vs baseline: 1.5983x; 1.5983x over previous
"""Trainium2 Bass kernel for DFBNet SSP (sparse_attention).

Data-parallel over batch: 8 samples -> 8 NeuronCores, one sample per core.

Per-sample device computation (the O(C*N^2) heavy work):
  - column norms of feature_q -> cn = fq / |fq_col|          (unit columns)
  - sim = cn.T @ cn                                          [N,N] gram matmul
  - T[k,n] = wb[k] * exp(2*sim[k,n])   (mask folded into Exp bias: -BIG)
  - BP1'[c,n] = sum_k (fq[c,k] + bgs[c]) * T[k,n]
      == colsum[n] * (bg_local[c,n] + bgs[c])   -- the softmax denominator
      colsum is a positive per-column scale that cancels in the final
      cosine, so no colsum/normalization pass is needed at all.
  - out_bg[n] = 10 * dot(BP1'_n, cn_n) / |BP1'_n|
  - out_fg[n] = dot(fp1s, cn_n)        with fp1s = 10 * FP1 / |FP1|

Host side computes only O(C+N) / O(C*N) prototype vectors (replicating the
reference pred chain in float64, incl. the discrete top-k fallback): the
{0,1} selection weights -> Exp bias column, FP1 (pre-normalized), and
bgs = (3/7)*bg_proto. All O(N^2) attention work stays on device.

bf16 is used for matmul operands (cn, T, fqT) -- PE streams at the same
1 cycle/row as f32r but weight loads halve, and DVE elementwise ops avoid
the slow f32r write path. Tolerance is 2e-2; measured error stays ~1e-3.
"""

import numpy as np

B, C, H, W = 8, 512, 32, 32
N = H * W
FG_THRES, BG_THRES, TOPK = 0.7, 0.6, 12
BIG = 60000.0

CC = C // 128  # 4 channel chunks
KC = N // 128  # 8 pixel chunks
NB = N // 512  # 2 psum-bank column groups

_cache = {}
_EYE = np.eye(128, dtype=np.float32)


# --------------------------------------------------------------------------
# host: selection weights + prototype vectors (exact reference semantics)
# --------------------------------------------------------------------------
def _host_prep(feature_q, support_feat, support_mask):
    fq = feature_q.astype(np.float64).reshape(B, C, N)
    sf = support_feat.astype(np.float64).reshape(B, C, N)
    mf = (support_mask.reshape(B, N) == 1).astype(np.float64)
    mb = 1.0 - mf
    FP = (sf * mf[:, None]).sum(-1) / (mf.sum(-1)[:, None] + 1e-5)
    BP = (sf * mb[:, None]).sum(-1) / (mb.sum(-1)[:, None] + 1e-5)

    def cos(a, b):  # a [B,C,N], b [B,C]
        dot = (a * b[:, :, None]).sum(1)
        na = np.sqrt((a * a).sum(1))
        nb = np.sqrt((b * b).sum(1))[:, None]
        return dot / np.maximum(na * nb, 1e-8)

    sfg = cos(fq, FP) * 10.0
    sbg = cos(fq, BP) * 10.0
    m = np.maximum(sfg, sbg)
    efg = np.exp(sfg - m)
    ebg = np.exp(sbg - m)
    pfg = efg / (efg + ebg)
    pbg = ebg / (efg + ebg)

    def select(pred, thres):
        w = np.zeros((B, N), np.float64)
        for b in range(B):
            row = pred[b] > thres
            if row.sum() > 0:
                w[b] = row
            else:
                # jax.lax.top_k tie-break: lower index wins -> stable argsort
                idx = np.argsort(-pred[b], kind="stable")[:TOPK]
                w[b, idx] = 1.0
        return w

    wf = select(pfg, FG_THRES)
    wb = select(pbg, BG_THRES)

    fg_proto = (fq * wf[:, None]).sum(-1) / wf.sum(-1)[:, None]  # [B,C]
    bg_proto = (fq * wb[:, None]).sum(-1) / wb.sum(-1)[:, None]
    FP1 = 0.5 * FP + 0.5 * fg_proto
    fp1s = 10.0 * FP1 / np.linalg.norm(FP1, axis=1, keepdims=True)  # [B,C]
    bgs = (3.0 / 7.0) * bg_proto  # [B,C]
    # Exp bias: {wb=0 -> -BIG, wb=1 -> 0}, laid out [128, KC] column-major
    biascol = ((wb - 1.0) * BIG).reshape(B, KC, 128).transpose(0, 2, 1)
    return (
        fp1s.reshape(B, CC, 128).transpose(0, 2, 1).astype(np.float32),  # [B,128,CC]
        bgs.astype(np.float32),  # [B,C] (row layout for broadcast)
        np.ascontiguousarray(biascol).astype(np.float32),  # [B,128,KC]
    )


# --------------------------------------------------------------------------
# device program
# --------------------------------------------------------------------------
def _make_tile_context_cls():
    import concourse.tile as tile
    from concourse.vector_clock import ScopedClock, VectorClock

    class PatchedTileContext(tile.TileContext):
        """This walrus build rejects CTRL/Drain instructions carrying more
        than one sem wait.  Put the tail-drain's global-clock waits on
        single-wait NOPs (same engine, program order) instead."""

        def _drain_and_barrier(self, tick_clock, wait_clock):
            gc = tick_clock.global_clock
            n = len(gc)
            for proc in range(n):
                t = gc[proc]
                if t > 0:
                    vec = [0] * n
                    vec[proc] = t
                    nop = self.nc.sync.nop(nofuse=True)
                    wait_clock.add_sem_waits(
                        nop.ins, ScopedClock({None: VectorClock(vec)})
                    )
            self.nc.sync.drain()
            self.nc.all_engine_barrier()
            assert self.sems is not None
            popped = self.nc._tile_sem_poison_stack.pop()
            assert popped is self._sem_poison
            self.nc.clear_and_free_semaphores(list(self.sems.allocated().values()))
            self.nc.all_engine_barrier()

    return PatchedTileContext


def _split_multi_waits(nc):
    """This walrus build allows at most one sync-wait command per
    instruction.  Move extra waits onto same-engine NOPs inserted just
    before the instruction (waits are AND conditions; order-safe)."""
    import concourse.mybir as mybir

    n_split = 0
    for f in nc.m.functions:
        for bb in f.blocks:
            il = bb.instructions
            i = 0
            while i < len(il):
                inst = il[i]
                si = inst.sync_info
                if si is not None and si.on_wait and len(si.on_wait) > 1:
                    waits = list(si.on_wait)
                    for j, w in enumerate(waits[:-1]):
                        nop = mybir.InstNoOp(
                            name=f"{inst.name}-wsplit{j}",
                            ins=[],
                            outs=[],
                            engine=inst.engine,
                            sync_info=mybir.SyncInfo(on_wait=[w], on_update=[]),
                        )
                        il.insert(i, nop)
                        i += 1
                        n_split += 1
                    inst.sync_info = mybir.SyncInfo(
                        on_wait=[waits[-1]], on_update=si.on_update
                    )
                i += 1
    return n_split


def _build_nc(split_waits=True):
    import concourse.bass as bass
    import concourse.mybir as mybir

    fp32 = mybir.dt.float32
    bf16 = mybir.dt.bfloat16
    AF = mybir.ActivationFunctionType

    PatchedTileContext = _make_tile_context_cls()

    nc = bass.Bass("TRN2", target_bir_lowering=False)
    fq_d = nc.declare_dram_parameter("fq", [C, N], fp32, isOutput=False)
    id_d = nc.declare_dram_parameter("ident", [128, 128], fp32, isOutput=False)
    bias_d = nc.declare_dram_parameter("biascol", [128, KC], fp32, isOutput=False)
    fp1s_d = nc.declare_dram_parameter("fp1s", [128, CC], fp32, isOutput=False)
    bgsr_d = nc.declare_dram_parameter("bgsrow", [1, C], fp32, isOutput=False)
    out_d = nc.declare_dram_parameter("out", [2, N], fp32, isOutput=True)

    def nbs(nb):
        return slice(nb * 512, (nb + 1) * 512)

    def cs(i):
        return slice(i * 128, (i + 1) * 128)

    with PatchedTileContext(nc) as tc:
        with (
            tc.tile_pool(name="consts", bufs=1) as consts,
            tc.tile_pool(name="big", bufs=1) as big,
            tc.tile_pool(name="scr", bufs=2) as scr,
            tc.tile_pool(name="small", bufs=1) as small,
        ):
            # ---- DMAs (big first so fq streams in behind the smalls)
            fq = []
            for cc in range(CC):
                t = big.tile([128, N], fp32, tag=f"fq{cc}", name=f"fqs{cc}")
                nc.sync.dma_start(t, fq_d[cs(cc), :])
                fq.append(t)
            ident = consts.tile([128, 128], fp32, tag="ident")
            nc.sync.dma_start(ident, id_d[:, :])
            biascol = consts.tile([128, KC], fp32, tag="biascol")
            nc.sync.dma_start(biascol, bias_d[:, :])
            fp1s = consts.tile([128, CC], fp32, tag="fp1s")
            nc.sync.dma_start(fp1s, fp1s_d[:, :])
            bgsrow = consts.tile([1, C], fp32, tag="bgsrow")
            nc.sync.dma_start(bgsrow, bgsr_d[:, :])

            ones_b = consts.tile([128, 128], bf16, tag="ones_b")
            nc.vector.memset(ones_b, 1.0)
            ones_r = consts.tile([1, 128], fp32, tag="ones_r")
            nc.vector.memset(ones_r, 1.0)
            fp1s_b = consts.tile([128, CC], bf16, tag="fp1s_b")
            nc.vector.tensor_copy(fp1s_b, fp1s)

            # ---- squares for column norms (Act; fq chunks stream in)
            sq = []
            for cc in range(CC):
                t = big.tile([128, N], bf16, tag=f"sq{cc}", name=f"sqs{cc}")
                nc.scalar.activation(t, fq[cc], AF.Square)
                sq.append(t)

            # persistent SBUF tensors (rnormB stays fp32: its per-column scale
            # error would hit both output rows un-cancelled; the cn multiply
            # runs at fp32 DVE rate regardless since fq is fp32)
            rnormB = big.tile([128, N], fp32, tag="rnormB")
            cn = [big.tile([128, N], bf16, tag=f"cn{cc}", name=f"cns{cc}") for cc in range(CC)]
            fqT = [big.tile([128, C], bf16, tag=f"fqT{kc}", name=f"fqTs{kc}") for kc in range(KC)]
            T = [big.tile([128, N], bf16, tag=f"T{kc}", name=f"Ts{kc}") for kc in range(KC)]
            bgsB = consts.tile([128, C], fp32, tag="bgsB")
            outfg = small.tile([1, N], fp32, tag="outfg")

            # fin: single PSUM bank; fg dot results at partition rows 0 / 32
            # (matmul outputs may only start at partition 0, 32 or 64)
            with tc.tile_pool(name="ps_fin", bufs=1, space="PSUM") as ps_fin:
                fin = ps_fin.tile([64, 512], fp32, tag="fin")

                with tc.tile_pool(name="ps_pre", bufs=1, space="PSUM") as ps_pre:
                    # bgs broadcast [128, C] via K=1 ones-matmul (fp32, one-off)
                    bps = ps_pre.tile([128, C], fp32, tag="bps")
                    nc.tensor.matmul(bps, ones_r, bgsrow, start=True, stop=True)
                    nc.scalar.copy(bgsB, bps)

                    # na2 (column norms^2) broadcast to all partitions via
                    # all-ones matmul; nb=0 first so gram can start early
                    n2ps = ps_pre.tile([128, N], fp32, tag="n2ps")
                    for nb in range(NB):
                        for cc in range(CC):
                            nc.tensor.matmul(
                                n2ps[:, nbs(nb)],
                                ones_b,
                                sq[cc][:, nbs(nb)],
                                start=(cc == 0),
                                stop=(cc == CC - 1),
                            )
                    # rnorm = na2^-0.5 ; cn = fq * rnorm  (per nb-half)
                    for nb in range(NB):
                        lnt = scr.tile([128, 512], fp32, tag="lnt", name=f"lnt{nb}")
                        nc.scalar.activation(lnt, n2ps[:, nbs(nb)], AF.Ln)
                        nc.scalar.activation(
                            rnormB[:, nbs(nb)], lnt, AF.Exp, scale=-0.5
                        )
                        for cc in range(CC):
                            nc.vector.tensor_mul(
                                cn[cc][:, nbs(nb)], fq[cc][:, nbs(nb)], rnormB[:, nbs(nb)]
                            )

                with tc.tile_pool(name="ps_r", bufs=2, space="PSUM") as ps_r:
                    R = [None] * CC  # recon psums per channel chunk

                    def gram(mi, ps_g):
                        gt = [
                            ps_g.tile([128, 512], fp32, tag="g", name=f"g{mi}_{nb}")
                            for nb in range(NB)
                        ]
                        for cc in range(CC):
                            for nb in range(NB):
                                nc.tensor.matmul(
                                    gt[nb],
                                    cn[cc][:, cs(mi)],
                                    cn[cc][:, nbs(nb)],
                                    start=(cc == 0),
                                    stop=(cc == CC - 1),
                                )
                        for nb in range(NB):
                            nc.scalar.activation(
                                T[mi][:, nbs(nb)],
                                gt[nb],
                                AF.Exp,
                                bias=biascol[:, mi : mi + 1],
                                scale=2.0,
                            )

                    def recon_a(kc, m2=0):
                        for nb in range(NB):
                            nc.tensor.matmul(
                                R[m2][:, nbs(nb)],
                                fqT[kc][:, cs(m2)],
                                T[kc][:, nbs(nb)],
                                start=(kc == 0),
                                stop=(kc == KC - 1),
                            )

                    with (
                        tc.tile_pool(name="ps_tr", bufs=1, space="PSUM") as ps_tr,
                        tc.tile_pool(name="ps_g", bufs=2, space="PSUM") as ps_g,
                    ):
                        # ---- main loop: gram(mi) + transpose(kc=mi) +
                        #      exp(mi) + deferred recon-A(mi-2) + fg dots

                        def transpose_kc(kc):
                            trp = ps_tr.tile([128, C], fp32, tag="trp", name=f"trp{kc}")
                            for cc in range(CC):
                                nc.tensor.transpose(
                                    trp[:, cs(cc)], fq[cc][:, cs(kc)], ident
                                )
                            # fqT = fq^T + bgs (broadcast along pixels)
                            nc.vector.tensor_add(fqT[kc], trp, bgsB)

                        def fg_dot(cc):
                            for nb in range(NB):
                                nc.tensor.matmul(
                                    fin[nb * 32 : nb * 32 + 1, :],
                                    fp1s_b[:, cc : cc + 1],
                                    cn[cc][:, nbs(nb)],
                                    start=(cc == 0),
                                    stop=(cc == CC - 1),
                                    skip_group_check=True,
                                )

                        R[0] = ps_r.tile([128, N], fp32, tag="r", name="R0")
                        for mi in range(KC):
                            gram(mi, ps_g)
                            transpose_kc(mi)
                            if mi >= 2:
                                recon_a(mi - 2)
                            if mi < CC:
                                fg_dot(mi)
                        recon_a(KC - 2)
                        recon_a(KC - 1)

                    # fg output row (ready as soon as fg dots retire)
                    for nb in range(NB):
                        nc.vector.tensor_copy(
                            outfg[:, nbs(nb)], fin[nb * 32 : nb * 32 + 1, :]
                        )
                    nc.sync.dma_start(out_d[1:2, :], outfg)

                    # ---- recon passes for remaining channel chunks + p/q
                    #      contractions (PE recon kept ahead of fin dots).
                    #      pqt[nb]: row 0 = p (BP1'.cn), row 32 = q (|BP1'|^2)
                    with tc.tile_pool(name="ps_pq", bufs=1, space="PSUM") as ps_pq:
                        pqt = [
                            ps_pq.tile([64, 512], fp32, tag=f"pq{nb}", name=f"pq{nb}")
                            for nb in range(NB)
                        ]

                        def pq(m2):
                            p_t = scr.tile([128, N], bf16, tag="p_t", name=f"p_t{m2}")
                            nc.vector.tensor_mul(p_t, cn[m2], R[m2])
                            q_t = scr.tile([128, N], bf16, tag="q_t", name=f"q_t{m2}")
                            nc.scalar.activation(q_t, R[m2], AF.Square)
                            return p_t, q_t

                        def fin_dots(m2, p_t, q_t):
                            for row, src in ((0, p_t), (32, q_t)):
                                for nb in range(NB):
                                    nc.tensor.matmul(
                                        pqt[nb][row : row + 1, :],
                                        ones_b[:, 0:1],
                                        src[:, nbs(nb)],
                                        start=(m2 == 0),
                                        stop=(m2 == CC - 1),
                                        skip_group_check=True,
                                    )

                        pqs = {}
                        for m2 in range(1, CC):
                            R[m2] = ps_r.tile([128, N], fp32, tag="r", name=f"R{m2}")
                            for kc in range(KC):
                                recon_a(kc, m2)
                            pqs[m2 - 1] = pq(m2 - 1)
                            fin_dots(m2 - 1, *pqs[m2 - 1])
                        pqs[CC - 1] = pq(CC - 1)
                        fin_dots(CC - 1, *pqs[CC - 1])

                        # ---- bg output row: 10 * p / sqrt(q)
                        pb = small.tile([1, N], fp32, tag="pb")
                        qb = small.tile([1, N], fp32, tag="qb")
                        for nb in range(NB):
                            nc.vector.tensor_copy(pb[:, nbs(nb)], pqt[nb][0:1, :])
                            nc.vector.tensor_copy(qb[:, nbs(nb)], pqt[nb][32:33, :])
                        rq = small.tile([1, N], fp32, tag="rq")
                        nc.vector.reciprocal(rq, qb)
                        sq10 = small.tile([1, N], fp32, tag="sq10")
                        nc.scalar.activation(sq10, rq, AF.Sqrt, scale=100.0)
                        outbg = small.tile([1, N], fp32, tag="outbg")
                        nc.vector.tensor_mul(outbg, pb, sq10)
                        nc.sync.dma_start(out_d[0:1, :], outbg)

    if split_waits:
        _split_multi_waits(nc)
    return nc


def _get_nc():
    if "nc" not in _cache:
        _cache["nc"] = _build_nc()
    return _cache["nc"]


def _make_in_maps(feature_q, support_feat, support_mask):
    fp1s, bgs, biascol = _host_prep(feature_q, support_feat, support_mask)
    fqr = np.ascontiguousarray(feature_q.reshape(B, C, N), dtype=np.float32)
    return [
        {
            "fq": fqr[b],
            "ident": _EYE,
            "biascol": biascol[b],
            "fp1s": fp1s[b],
            "bgsrow": bgs[b : b + 1],
        }
        for b in range(B)
    ]


def run_sharded(feature_q, support_feat, support_mask, **kwargs):
    """Run on all 8 cores; returns (output [B,2,H,W], BassKernelResults)."""
    from concourse.bass_utils import run_bass_kernel_spmd

    nc = _get_nc()
    in_maps = _make_in_maps(feature_q, support_feat, support_mask)
    res = run_bass_kernel_spmd(nc, in_maps, core_ids=list(range(B)), **kwargs)
    out = np.stack([res.results[b]["out"] for b in range(B)])
    return out.reshape(B, 2, H, W).astype(np.float32), res


def kernel(feature_q, support_feat, support_mask):
    out, _ = run_sharded(
        np.asarray(feature_q), np.asarray(support_feat), np.asarray(support_mask)
    )
    return out
